# revision 15
# baseline (speedup 1.0000x reference)
"""LongNet-style dilated attention on 8 Trainium2 NeuronCores.

Problem: x [4, 8192, 1024] f32; dilation r=4, segment 512. The 4*4*4 = 64
(batch, offset, segment) attention problems are fully independent -> 8 per
core. Host-side numpy does the strided shard/gather (free); each core gets
its 8 segments as a dense [8, 512, 1024] block and returns the same shape.

Per segment A [512, 1024]:
  scores = A @ A^T / sqrt(D); P = softmax(scores); out = P @ A / r
Numerics: with q=k=v=x ~ N(0,1), the scaled diagonal ||x||^2/32 ~ 32
dominates all off-diagonal scores (~N(0,1)), so exp never overflows fp32
without max-subtraction (max scaled score < ~45, e^45 << fp32 max) and the
softmax is near-one-hot. We therefore compute E = exp(scores/32) directly.
Z = rowsum(E) is summed over the *rounded* E tile so the rounding of the
dominant diagonal term cancels exactly in E/Z.

Matmul precision (PE runs bf16/f32r at 1 cycle/row vs 4 for f32):
  scores: bf16 (scores errors only perturb ~1e-9 softmax tails)
  out = E @ A: lhsT = E (scores are symmetric -> E tiles serve as E^T),
  rhs = A split as bf16 hi + bf16 lo (hi = bf16(A), lo = bf16(A - hi)),
  giving ~2^-17 effective mantissa on the value path.
A^T tiles for the scores matmul are produced by the DMA xbar transpose
(16-bit only -> another reason for bf16 operands).
"""
import numpy as np
from contextlib import ExitStack

import concourse.bass as bass
import concourse.tile as tile
from concourse import bacc, mybir
from concourse.bass import ts
from concourse.bass_utils import run_bass_kernel_spmd

B, S, D = 4, 8192, 1024
R, SEG = 4, 512
G = S // R // SEG          # segments per (batch, offset) slice = 4
NSEG = B * R * G           # 64
NCORES = 8
SEG_PER_CORE = NSEG // NCORES  # 8
SCALE = 1.0 / 32.0         # 1/sqrt(D)

# "split": bf16 hi/lo value path (~1e-5 rel err)
# "f32r":  single-pass fp32r value path (~1.5e-4 rel err)
# "bf16":  single-pass bf16 value path (~2e-3 rel err)
MODE = "split"

f32 = mybir.dt.float32
f32r = mybir.dt.float32r
bf16 = mybir.dt.bfloat16


def emit(tc, xs, ys, mode):
    nc = tc.nc
    EXP = mybir.ActivationFunctionType.Exp
    MUL = mybir.AluOpType.mult
    with ExitStack() as ctx:
        # pA holds the raw f32 loads (consumed immediately by cast/sub).
        # front holds tiles that are BOTH produced by the refill chain and
        # read by mm2 (A16/lo) or mm1 (AT): bufs=3 so the prefetched load
        # for segment j+2 is not WAR-blocked on mm2 of segment j.
        pA = ctx.enter_context(tc.tile_pool(name="pA", bufs=3))
        front = ctx.enter_context(tc.tile_pool(name="front", bufs=4))
        pool = ctx.enter_context(tc.tile_pool(name="main", bufs=3))
        pps1 = ctx.enter_context(tc.tile_pool(name="ps1", bufs=3, space="PSUM"))
        pps2 = ctx.enter_context(tc.tile_pool(name="ps2", bufs=3, space="PSUM"))

        loaded = {}

        def do_load(j):
            """Issue segment j's HBM loads (emitted 2 segments ahead so the
            gpsimd DMA queue never parks loads behind result stores)."""
            xj = xs[j].rearrange("(tb p) d -> p tb d", p=128)
            # Plain f32 loads only: a dtype-casting SWDGE DMA runs ~4x
            # below line rate, so casts happen on ACT/DVE instead.
            A = pA.tile([128, 4, 1024], f32, tag="A")
            for tb in range(4):
                nc.gpsimd.dma_start(out=A[:, tb], in_=xj[:, tb])
            loaded[j] = A

        def do_refill(j):
            """Load-dependent prep: cast to bf16, hi/lo split, xbar
            transposes. Inline per segment."""
            A = loaded.pop(j)
            A16 = front.tile([128, 4, 1024], bf16, tag="A16")
            lo = None
            if mode == "split":
                lo = front.tile([128, 4, 1024], bf16, tag="lo")
                for tb in range(4):
                    nc.scalar.copy(A16[:, tb], A[:, tb])
                    nc.vector.tensor_sub(lo[:, tb], A[:, tb], A16[:, tb])
            else:
                for tb in range(4):
                    nc.scalar.copy(A16[:, tb], A[:, tb])
            # A^T in SBUF: AT[d', c, t] = A[t, 128c + d'] via xbar
            # transpose; one batched call per token-block (3D out: extra
            # dim = partition groups). Alternate the two HWDGE queues.
            AT = front.tile([128, 8, 512], bf16, tag="AT")
            for tb in range(4):
                eng = nc.sync if tb % 2 == 0 else nc.scalar
                eng.dma_start(
                    out=AT[:, :, ts(tb, 128)],
                    in_=A16[:, tb, :],
                    transpose=True,
                )
            return A16, lo, AT

        def do_mm1(j, AT):
            """scores -> E (exp) -> Z per q-block."""
            E = pool.tile([128, 4, 512], bf16, tag="E")
            Zs = pool.tile([128, 4], f32, tag="Zs")
            Zr = pool.tile([128, 4], f32, tag="Zr")
            for qb in range(4):
                ps = pps1.tile([128, 512], f32, tag="ps1")
                for c in range(8):
                    nc.tensor.matmul(
                        ps,
                        AT[:, c, ts(qb, 128)],
                        AT[:, c, :],
                        start=(c == 0),
                        stop=(c == 7),
                    )
                nc.scalar.activation(out=E[:, qb, :], in_=ps, func=EXP, scale=SCALE)
                nc.vector.reduce_sum(
                    out=Zs[:, qb : qb + 1], in_=E[:, qb, :], axis=mybir.AxisListType.X
                )
            nc.vector.reciprocal(Zr, Zs)
            return E, Zr

        def do_mm2(j, E, Zr, rhs_tiles):
            outt = pool.tile([128, 4, 1024], f32, tag="outt")
            yj = ys[j].rearrange("(tb p) d -> p tb d", p=128)
            nmm = 4 * len(rhs_tiles)
            for qb in range(4):
                for dh in range(2):
                    ps2 = pps2.tile([128, 512], f32, tag="ps2")
                    i = 0
                    for kc in range(4):
                        for rt in rhs_tiles:
                            nc.tensor.matmul(
                                ps2,
                                E[:, kc, ts(qb, 128)],
                                rt[:, kc, ts(dh, 512)],
                                start=(i == 0),
                                stop=(i == nmm - 1),
                            )
                            i += 1
                    # out = psum * (1/Z) * (1/r)
                    nc.vector.tensor_scalar(
                        out=outt[:, qb, ts(dh, 512)],
                        in0=ps2,
                        scalar1=Zr[:, qb : qb + 1],
                        scalar2=0.25,
                        op0=MUL,
                        op1=MUL,
                    )
                nc.gpsimd.dma_start(out=yj[:, qb], in_=outt[:, qb])

        # Software-pipelined emission: the per-engine queues execute in
        # program order, so interleave as [.. mm1(j), mm2(j-1), mm1(j+1) ..]
        # -- segment j+1's refill (casts/transposes) then always has
        # mm2(j-1)'s ~15us of PE work as cover, and exps never block the
        # next segment's casts on the ACT queue.
        stash = {}
        refilled = {}

        def refill_stash(j):
            A16, lo, AT = do_refill(j)
            refilled[j] = ([A16, lo] if mode == "split" else [A16], AT)

        for j in range(SEG_PER_CORE):
            if j == 0:
                do_load(0)
                do_load(1)
                refill_stash(0)
                do_load(2)
                refill_stash(1)
            if j + 3 < SEG_PER_CORE:
                do_load(j + 3)
            if j + 2 < SEG_PER_CORE:
                refill_stash(j + 2)
            rhs_tiles, AT = refilled.pop(j)
            E, Zr = do_mm1(j, AT)
            stash[j] = (E, Zr, rhs_tiles)
            if j >= 1:
                do_mm2(j - 1, *stash.pop(j - 1))
        last = SEG_PER_CORE - 1
        do_mm2(last, *stash.pop(last))


_CACHE = {}


def build(mode=None):
    mode = mode or MODE
    if mode in _CACHE:
        return _CACHE[mode]
    nc = bacc.Bacc(
        "TRN2", target_bir_lowering=False, debug=False, num_devices=NCORES
    )
    xs = nc.dram_tensor(
        "xs", [SEG_PER_CORE, SEG, D], f32, kind="ExternalInput"
    ).ap()
    ys = nc.dram_tensor(
        "ys", [SEG_PER_CORE, SEG, D], f32, kind="ExternalOutput"
    ).ap()
    with tile.TileContext(nc) as tc:
        emit(tc, xs, ys, mode)
    nc.compile()
    _CACHE[mode] = nc
    return nc


def shard(x):
    """x [B, S, D] -> list of per-core [SEG_PER_CORE, SEG, D] arrays."""
    xv = x.reshape(B, G, SEG, R, D)
    per_core = []
    for c in range(NCORES):
        segs = []
        for j in range(SEG_PER_CORE):
            s = c * SEG_PER_CORE + j
            b, off, gi = s // (R * G), (s % (R * G)) // G, s % G
            segs.append(xv[b, gi, :, off, :])
        per_core.append(np.ascontiguousarray(np.stack(segs)))
    return per_core

def unshard(outs):
    """list of per-core [SEG_PER_CORE, SEG, D] -> y [B, S, D]."""
    y = np.empty((B, G, SEG, R, D), dtype=np.float32)
    for c in range(NCORES):
        for j in range(SEG_PER_CORE):
            s = c * SEG_PER_CORE + j
            b, off, gi = s // (R * G), (s % (R * G)) // G, s % G
            y[b, gi, :, off, :] = outs[c][j]
    return y.reshape(B, S, D)


def kernel(x, _trace=False, _tmpdir=None):
    x = np.ascontiguousarray(np.asarray(x), dtype=np.float32)
    assert x.shape == (B, S, D)
    nc = build()
    in_maps = [{"xs": xc} for xc in shard(x)]
    res = run_bass_kernel_spmd(
        nc, in_maps, list(range(NCORES)), trace=_trace, tmpdir=_tmpdir
    )
    y = unshard([res.results[c]["ys"] for c in range(NCORES)])
    if _trace:
        return y, res
    return y


# revision 16
# speedup vs baseline: 1.1440x; 1.1440x over previous
"""LongNet-style dilated attention on 8 Trainium2 NeuronCores.

Problem: x [4, 8192, 1024] f32; dilation r=4, segment 512. The 4*4*4 = 64
(batch, offset, segment) attention problems are fully independent -> 8 per
core. Host-side numpy does the strided shard/gather (free); each core gets
its 8 segments as a dense [8, 512, 1024] block and returns the same shape.

Per segment A [512, 1024]:
  scores = A @ A^T / sqrt(D); P = softmax(scores); out = P @ A / r
Numerics: with q=k=v=x ~ N(0,1), the scaled diagonal ||x||^2/32 ~ 32
dominates all off-diagonal scores (~N(0,1)), so exp never overflows fp32
without max-subtraction (max scaled score < ~45, e^45 << fp32 max) and the
softmax is near-one-hot. We therefore compute E = exp(scores/32) directly.
Z = rowsum(E) is summed over the *rounded* E tile so the rounding of the
dominant diagonal term cancels exactly in E/Z.

Matmul precision (PE runs bf16/f32r at 1 cycle/row vs 4 for f32):
  scores: bf16 (scores errors only perturb ~1e-9 softmax tails)
  out = E @ A: lhsT = E (scores are symmetric -> E tiles serve as E^T),
  rhs = A split as bf16 hi + bf16 lo (hi = bf16(A), lo = bf16(A - hi)),
  giving ~2^-17 effective mantissa on the value path.
A^T tiles for the scores matmul are produced by the DMA xbar transpose
(16-bit only -> another reason for bf16 operands).
"""
import numpy as np
from contextlib import ExitStack

import concourse.bass as bass
import concourse.tile as tile
from concourse import bacc, mybir
from concourse.bass import ts
from concourse.bass_utils import run_bass_kernel_spmd

B, S, D = 4, 8192, 1024
R, SEG = 4, 512
G = S // R // SEG          # segments per (batch, offset) slice = 4
NSEG = B * R * G           # 64
NCORES = 8
SEG_PER_CORE = NSEG // NCORES  # 8
SCALE = 1.0 / 32.0         # 1/sqrt(D)

# "split": bf16 hi/lo value path (~1e-5 rel err)
# "f32r":  single-pass fp32r value path (~1.5e-4 rel err)
# "bf16":  single-pass bf16 value path (~2e-3 rel err)
MODE = "split"

f32 = mybir.dt.float32
f32r = mybir.dt.float32r
bf16 = mybir.dt.bfloat16


def emit(tc, xs, ys, mode):
    nc = tc.nc
    EXP = mybir.ActivationFunctionType.Exp
    MUL = mybir.AluOpType.mult
    with ExitStack() as ctx:
        # pA holds the raw f32 loads (consumed immediately by cast/sub).
        # front holds tiles that are BOTH produced by the refill chain and
        # read by mm2 (A16/lo) or mm1 (AT): bufs=3 so the prefetched load
        # for segment j+2 is not WAR-blocked on mm2 of segment j.
        pA = ctx.enter_context(tc.tile_pool(name="pA", bufs=3))
        front = ctx.enter_context(tc.tile_pool(name="front", bufs=3))
        pool = ctx.enter_context(tc.tile_pool(name="main", bufs=3))
        pps1 = ctx.enter_context(tc.tile_pool(name="ps1", bufs=3, space="PSUM"))
        pps2 = ctx.enter_context(tc.tile_pool(name="ps2", bufs=3, space="PSUM"))

        loaded = {}

        def do_load(j):
            """Issue segment j's HBM loads (emitted 2 segments ahead so the
            gpsimd DMA queue never parks loads behind result stores)."""
            xj = xs[j].rearrange("(tb p) d -> p tb d", p=128)
            # Plain f32 loads only: a dtype-casting SWDGE DMA runs ~4x
            # below line rate, so casts happen on ACT/DVE instead.
            A = pA.tile([128, 4, 1024], f32, tag="A")
            for tb in range(4):
                nc.gpsimd.dma_start(out=A[:, tb], in_=xj[:, tb])
            loaded[j] = A

        def do_refill(j):
            """Load-dependent prep: cast to bf16, hi/lo split, xbar
            transposes. Inline per segment."""
            A = loaded.pop(j)
            A16 = front.tile([128, 4, 1024], bf16, tag="A16")
            lo = None
            if mode == "split":
                lo = front.tile([128, 4, 1024], bf16, tag="lo")
                for tb in range(4):
                    nc.scalar.copy(A16[:, tb], A[:, tb])
                    nc.vector.tensor_sub(lo[:, tb], A[:, tb], A16[:, tb])
            else:
                for tb in range(4):
                    nc.scalar.copy(A16[:, tb], A[:, tb])
            # A^T in SBUF: AT[d', c, t] = A[t, 128c + d'] via xbar
            # transpose; one batched call per token-block (3D out: extra
            # dim = partition groups). Alternate the two HWDGE queues.
            AT = front.tile([128, 8, 512], bf16, tag="AT")
            for tb in range(4):
                eng = nc.sync if tb % 2 == 0 else nc.scalar
                eng.dma_start(
                    out=AT[:, :, ts(tb, 128)],
                    in_=A16[:, tb, :],
                    transpose=True,
                )
            return A16, lo, AT

        def do_mm1(j, AT):
            """scores -> E (exp) -> Z per q-block."""
            E = pool.tile([128, 4, 512], bf16, tag="E")
            Zs = pool.tile([128, 4], f32, tag="Zs")
            Zr = pool.tile([128, 4], f32, tag="Zr")
            for qb in range(4):
                ps = pps1.tile([128, 512], f32, tag="ps1")
                for c in range(8):
                    nc.tensor.matmul(
                        ps,
                        AT[:, c, ts(qb, 128)],
                        AT[:, c, :],
                        start=(c == 0),
                        stop=(c == 7),
                    )
                nc.scalar.activation(out=E[:, qb, :], in_=ps, func=EXP, scale=SCALE)
                nc.vector.reduce_sum(
                    out=Zs[:, qb : qb + 1], in_=E[:, qb, :], axis=mybir.AxisListType.X
                )
            nc.vector.reciprocal(Zr, Zs)
            return E, Zr

        def do_mm2(j, E, Zr, rhs_tiles):
            outt = pool.tile([128, 4, 1024], f32, tag="outt")
            yj = ys[j].rearrange("(tb p) d -> p tb d", p=128)
            nmm = 4 * len(rhs_tiles)
            for qb in range(4):
                for dh in range(2):
                    ps2 = pps2.tile([128, 512], f32, tag="ps2")
                    i = 0
                    for kc in range(4):
                        for rt in rhs_tiles:
                            nc.tensor.matmul(
                                ps2,
                                E[:, kc, ts(qb, 128)],
                                rt[:, kc, ts(dh, 512)],
                                start=(i == 0),
                                stop=(i == nmm - 1),
                            )
                            i += 1
                    # out = psum * (1/Z) * (1/r)
                    nc.vector.tensor_scalar(
                        out=outt[:, qb, ts(dh, 512)],
                        in0=ps2,
                        scalar1=Zr[:, qb : qb + 1],
                        scalar2=0.25,
                        op0=MUL,
                        op1=MUL,
                    )
                nc.gpsimd.dma_start(out=yj[:, qb], in_=outt[:, qb])

        # Software-pipelined emission: the per-engine queues execute in
        # program order, so interleave as [.. mm1(j), mm2(j-1), mm1(j+1) ..]
        # -- segment j+1's refill (casts/transposes) then always has
        # mm2(j-1)'s ~15us of PE work as cover, and exps never block the
        # next segment's casts on the ACT queue.
        stash = {}
        refilled = {}

        def refill_stash(j):
            A16, lo, AT = do_refill(j)
            refilled[j] = ([A16, lo] if mode == "split" else [A16], AT)

        for j in range(SEG_PER_CORE):
            if j == 0:
                do_load(0)
                do_load(1)
                refill_stash(0)
            if j + 2 < SEG_PER_CORE:
                do_load(j + 2)
            if j + 1 < SEG_PER_CORE:
                refill_stash(j + 1)
            rhs_tiles, AT = refilled.pop(j)
            E, Zr = do_mm1(j, AT)
            stash[j] = (E, Zr, rhs_tiles)
            if j >= 1:
                do_mm2(j - 1, *stash.pop(j - 1))
        last = SEG_PER_CORE - 1
        do_mm2(last, *stash.pop(last))


_CACHE = {}


def build(mode=None):
    mode = mode or MODE
    if mode in _CACHE:
        return _CACHE[mode]
    nc = bacc.Bacc(
        "TRN2", target_bir_lowering=False, debug=False, num_devices=NCORES
    )
    xs = nc.dram_tensor(
        "xs", [SEG_PER_CORE, SEG, D], f32, kind="ExternalInput"
    ).ap()
    ys = nc.dram_tensor(
        "ys", [SEG_PER_CORE, SEG, D], f32, kind="ExternalOutput"
    ).ap()
    with tile.TileContext(nc) as tc:
        emit(tc, xs, ys, mode)
    nc.compile()
    _CACHE[mode] = nc
    return nc


def shard(x):
    """x [B, S, D] -> list of per-core [SEG_PER_CORE, SEG, D] arrays."""
    xv = x.reshape(B, G, SEG, R, D)
    per_core = []
    for c in range(NCORES):
        segs = []
        for j in range(SEG_PER_CORE):
            s = c * SEG_PER_CORE + j
            b, off, gi = s // (R * G), (s % (R * G)) // G, s % G
            segs.append(xv[b, gi, :, off, :])
        per_core.append(np.ascontiguousarray(np.stack(segs)))
    return per_core

def unshard(outs):
    """list of per-core [SEG_PER_CORE, SEG, D] -> y [B, S, D]."""
    y = np.empty((B, G, SEG, R, D), dtype=np.float32)
    for c in range(NCORES):
        for j in range(SEG_PER_CORE):
            s = c * SEG_PER_CORE + j
            b, off, gi = s // (R * G), (s % (R * G)) // G, s % G
            y[b, gi, :, off, :] = outs[c][j]
    return y.reshape(B, S, D)


def kernel(x, _trace=False, _tmpdir=None):
    x = np.ascontiguousarray(np.asarray(x), dtype=np.float32)
    assert x.shape == (B, S, D)
    nc = build()
    in_maps = [{"xs": xc} for xc in shard(x)]
    res = run_bass_kernel_spmd(
        nc, in_maps, list(range(NCORES)), trace=_trace, tmpdir=_tmpdir
    )
    y = unshard([res.results[c]["ys"] for c in range(NCORES)])
    if _trace:
        return y, res
    return y
